# revision 26
# baseline (speedup 1.0000x reference)
"""Trainium2 Bass kernel for CapsuleLayer (dynamic routing), 8-core data-parallel.

Problem: x [128, 1152, 512] f32, W [512, 160] f32.
  u_hat = (x @ W).reshape(B, N, 10, 16)
  b = 0; 3 routing iterations of softmax/weighted-sum/squash.
Output: v [128, 10, 16] f32.

Sharding: data-parallel over batch. Each of the 8 cores gets 16 batches
(x shard [16*1152, 512]) and the full W; no cross-core communication.

Per-core pipeline:
  Phase 1 (streamed over 144 row-tiles of 128):
    - DMA x tile f32 -> SBUF, cast to bf16 in the SWDGE DMA
    - transpose each [128,128] block on the PE as a regular bf16 matmul
      against an identity moving operand; PSUM->SBUF copies split across
      ACT / DVE / GPSIMD by load balance
    - u_hat tile = xT.T @ W accumulated over the 4 k-chunks in PSUM
  Phase 2 (on-chip routing, u_hat resident in SBUF):
    - every hot elementwise op is a scalar_tensor_tensor (tensor-scalar
      HW path: 2x/4x DVE perf modes) on bf16 SBUF operands
    - b logits and exp() kept in bf16; softmax denominator via a 5-2-1
      add tree (STT) instead of a slow 1x tensor_reduce
    - squash factor sqrt(n2)/(1+n2) computed as exp(0.5*ln(n2)-ln(1+n2))
      so the scalar engine only ever needs the {Exp, Ln, Copy, Square}
      activation table: ZERO table reloads in steady state
    - rt12_b capsule-sum matmuls packed two batches per slot via
      tile_position (col groups 0/32), replication matmuls row-packed
"""

import os
import sys

import numpy as np

sys.path.insert(0, "/opt/trn_rl_repo")

import concourse.bass as bass
import concourse.tile as tile
import concourse.mybir as mybir
from concourse import bacc

F32 = mybir.dt.float32
BF16 = mybir.dt.bfloat16
AX = mybir.AxisListType
ALU = mybir.AluOpType
ACTF = mybir.ActivationFunctionType

B, N, K, C, D = 128, 1152, 512, 10, 16
CD = C * D  # 160
NCORES = 8
BSH = B // NCORES  # batches per core

XB_BATCH = 4  # n-tiles per x-load DMA
DMA_T_MOD = 0  # if >0, every DMA_T_MOD-th tile transposes via the Sync xbar
ACT_TABLE = "natural_log_exp_and_others"  # covers Exp/Ln/Copy/Square/Identity


def build_core_program(bsh=BSH, n_len=N, nc=None):
    """Build the single-core Bass program for a shard of `bsh` batches."""
    assert n_len % 128 == 0
    tpb = n_len // 128  # row-tiles per batch
    rows = bsh * n_len

    if nc is None:
        nc = bacc.Bacc("TRN2", target_bir_lowering=False, debug=False)

    x_in = nc.declare_dram_parameter("x", [rows, K], F32, isOutput=False).ap()
    w_in = nc.declare_dram_parameter("W", [K, CD], F32, isOutput=False).ap()
    id_in = nc.declare_dram_parameter("ident", [128, 128], BF16, isOutput=False).ap()
    mk_in = nc.declare_dram_parameter("mask", [42, CD], BF16, isOutput=False).ap()
    v_out = nc.declare_dram_parameter("v", [bsh, CD], F32, isOutput=True).ap()

    with tile.TileContext(nc) as tc:
        _build_body(tc, x_in, w_in, id_in, mk_in, v_out, bsh, tpb)
    nc.finalize()
    return nc


def _stt(eng, out, in0, in1, op1, scalar=1.0, op0=ALU.mult):
    """out = (in0 * scalar) op1 in1 on the tensor-scalar HW path."""
    eng.scalar_tensor_tensor(out=out, in0=in0, scalar=scalar, in1=in1, op0=op0, op1=op1)


def _build_body(tc, x_in, w_in, id_in, mk_in, v_out, bsh, tpb):
    nc = tc.nc
    nt = bsh * tpb
    KT = K // 128  # 4 contraction chunks

    from contextlib import ExitStack

    with ExitStack() as ctx:
        singles = ctx.enter_context(tc.tile_pool(name="singles", bufs=1))
        persist = ctx.enter_context(tc.tile_pool(name="persist", bufs=1))
        pool_xb = ctx.enter_context(tc.tile_pool(name="xb", bufs=3))
        pool_xT = ctx.enter_context(tc.tile_pool(name="xT", bufs=8))
        pool_sm = ctx.enter_context(tc.tile_pool(name="smalls", bufs=6))
        ps_U = ctx.enter_context(tc.tile_pool(name="psU", bufs=2, space="PSUM"))
        ps_P = ctx.enter_context(tc.tile_pool(name="psP", bufs=1, space="PSUM"))
        ps_S = ctx.enter_context(tc.tile_pool(name="psS", bufs=3, space="PSUM"))
        ps_T = ctx.enter_context(tc.tile_pool(name="psT", bufs=2, space="PSUM"))

        # --- constants ---
        # Pre-load the one activation table that covers every ACT func used
        # here (Exp, Ln, Copy, Square, Identity). Without this the compiler
        # greedily picks exp_and_others for Exp and natural_log for Ln and
        # ping-pongs between them: 49 table loads x 1.28us measured.
        from concourse.hw_specs import get_activation_tables

        if ACT_TABLE:
            tab_id = list(get_activation_tables(nc.m.arch)).index(ACT_TABLE)
            nc.scalar.add_instruction(
                mybir.InstLoadActFuncSet(
                    name=nc.get_next_instruction_name(),
                    ins=[],
                    outs=[],
                    act_func_set_id=tab_id,
                )
            )

        ident = singles.tile([128, 128], BF16)
        nc.sync.dma_start(out=ident, in_=id_in)
        # mask rows 0-9 and 32-41 carry the per-capsule diagonal selector so
        # both col-group positions have an aligned copy
        mask = singles.tile([42, CD], BF16)
        nc.sync.dma_start(out=mask, in_=mk_in)
        ones_m = singles.tile([128, 128], BF16)
        nc.vector.memset(ones_m, 1.0)
        tenth_m = singles.tile([128, 128], BF16)
        nc.vector.memset(tenth_m, 1.0 / C)

        w_f32 = singles.tile([128, KT, CD], F32)
        nc.sync.dma_start(out=w_f32, in_=w_in.rearrange("(j p) c -> p j c", p=128))
        w_bf = singles.tile([128, KT, CD], BF16)
        nc.vector.tensor_copy(w_bf, w_f32)

        # --- persistent tensors ---
        u_hat = persist.tile([128, nt, CD], BF16)
        w_scr = persist.tile([128, nt, CD], BF16)
        b_log = persist.tile([128, nt * C], BF16)
        e_exp = persist.tile([128, nt * C], BF16)
        c_sm = persist.tile([128, nt * C], BF16)
        zscr = persist.tile([128, nt, 8], BF16)  # softmax sum tree scratch
        ssum = persist.tile([128, nt], F32)  # 1/Z
        sq_all = persist.tile([128, bsh, CD], F32)
        n2_all = persist.tile([128, bsh * C], F32)
        ln1_all = persist.tile([128, bsh * C], F32)
        g_all = persist.tile([128, bsh * C], F32)
        vrep_bf = persist.tile([128, bsh, CD], BF16)
        vrep = persist.tile([128, bsh, CD], F32)

        # views
        u4 = u_hat[:].rearrange("p (g t) c -> p g t c", g=bsh)
        w4 = w_scr[:].rearrange("p (g t) c -> p g t c", g=bsh)
        w5 = w_scr[:].rearrange("p t (c d) -> p t c d", d=D)
        b3 = b_log[:].rearrange("p (t c) -> p t c", c=C)
        e3 = e_exp[:].rearrange("p (t c) -> p t c", c=C)
        c3 = c_sm[:].rearrange("p (t c) -> p t c", c=C)

        # ---------------- Phase 1 emitters ----------------
        xb_cur = [None]
        pu_cur = [None]

        def emit_trans(t):
            tb, tt = divmod(t, XB_BATCH)
            if tt == 0:
                xb = pool_xb.tile([128, XB_BATCH, K], BF16, tag="xb")
                src = x_in[tb * XB_BATCH * 128 : (tb + 1) * XB_BATCH * 128, :]
                nc.gpsimd.dma_start(
                    out=xb, in_=src.rearrange("(t p) k -> p t k", p=128)
                )
                xb_cur[0] = xb
            xb = xb_cur[0]
            xt4 = pool_xT.tile([128, KT, 128], BF16, tag="xt4")
            if DMA_T_MOD > 0 and t % DMA_T_MOD == DMA_T_MOD - 1:
                nc.sync.dma_start_transpose(xt4, xb[:, tt, :])
            else:
                pt4 = ps_T.tile([128, KT, 128], F32, tag="psT4")
                for j in range(KT):
                    nc.tensor.matmul(
                        pt4[:, j, :],
                        lhsT=xb[:, tt, j * 128 : (j + 1) * 128],
                        rhs=ident,
                        start=True,
                        stop=True,
                    )
                # PSUM->SBUF copy: rotate DVE / ACT by tile index
                if t % 4 == 1:
                    nc.vector.tensor_copy(xt4, pt4)
                else:
                    nc.scalar.copy(xt4, pt4)
            return xt4

        UCP = 3 if tpb % 3 == 0 else (2 if tpb % 2 == 0 else 1)  # tiles per u-copy

        def emit_gemm(t, xt4):
            lt = t % tpb
            loc = lt % UCP
            if loc == 0 or pu_cur[0] is None:
                pu_cur[0] = ps_U.tile([128, UCP, CD], F32, tag="psU2", name="pu2")
            pu2 = pu_cur[0]
            for j in range(KT):
                nc.tensor.matmul(
                    pu2[:, loc, :],
                    lhsT=xt4[:, j, :],
                    rhs=w_bf[:, j, :],
                    start=(j == 0),
                    stop=(j == KT - 1),
                )
            if loc == UCP - 1 or lt == tpb - 1:
                dst = u_hat[:, t - loc : t + 1, :]
                srcp = pu2 if loc == UCP - 1 else pu2[:, 0 : loc + 1, :]
                # u_hat copies alternate DVE / ACT
                if (t // UCP) % 2 == 0:
                    nc.vector.tensor_copy(dst, srcp)
                else:
                    nc.scalar.copy(dst, srcp)
                pu_cur[0] = None

        # ---------------- Phase 2 (routing) emitters, per group ----------
        NG = 8 if bsh % 8 == 0 else (4 if bsh % 4 == 0 else bsh)
        GB = bsh // NG  # batches per group
        TG = GB * tpb  # tiles per group

        def squash_group(gr, sp, last):
            # v = s * g,  g = sqrt(n2)/(1+n2) = exp(0.5*ln(n2) - ln(1+n2))
            # (the +1e-7 of the reference only perturbs v by O(1e-7)).
            # s lives in PSUM (sp [128, GB, CD]) and is read from there.
            gs = slice(gr * GB, (gr + 1) * GB)
            cs = slice(gr * GB * C, (gr + 1) * GB * C)
            sq_g = sq_all[:, gs, :]
            n2_g = n2_all[:, cs]
            ln1_g = ln1_all[:, cs]
            g_g = g_all[:, cs]
            nc.scalar.square(sq_g, sp)
            nc.vector.tensor_reduce(
                n2_g,
                sq_g.rearrange("p g (c d) -> p (g c) d", d=D),
                axis=AX.X,
                op=ALU.add,
            )
            nc.scalar.activation(ln1_g, n2_g, ACTF.Ln, bias=1.0)  # ln(1+n2)
            nc.scalar.activation(g_g, n2_g, ACTF.Ln)  # ln(n2)
            # g = exp(0.5*ln(n2) - ln(1+n2))
            _stt(nc.vector, g_g, g_g, ln1_g, ALU.subtract, scalar=0.5)
            nc.scalar.activation(g_g, g_g, ACTF.Exp)
            fb = g_g.broadcast_to([128, GB * C, D])
            out = (vrep if last else vrep_bf)[:, gs, :]
            nc.vector.tensor_mul(
                out.rearrange("p g (c d) -> p (g c) d", d=D),
                sp.rearrange("p g (c d) -> p (g c) d", d=D),
                fb,
            )
            if last:
                nc.sync.dma_start(
                    out=v_out[gr * GB : (gr + 1) * GB, :], in_=vrep[0:1, gs, :]
                )

        def rt0(gr):
            # s0 = 0.1 * sum_{n,t} u_hat per batch, on the PE
            sp = ps_S.tile([128, GB, CD], F32, tag="psS")
            for j, g in enumerate(range(gr * GB, (gr + 1) * GB)):
                for tt in range(tpb):
                    t = g * tpb + tt
                    nc.tensor.matmul(
                        sp[:, j, :],
                        lhsT=tenth_m,
                        rhs=u_hat[:, t, :],
                        start=(tt == 0),
                        stop=(tt == tpb - 1),
                    )
            squash_group(gr, sp, last=False)

        def rt12_a(gr, i):
            gs = slice(gr * GB, (gr + 1) * GB)
            ts = slice(gr * TG, (gr + 1) * TG)
            # b update: b (+)= sum_d u_hat * v_prev (bf16 tensor_tensor, which
            # runs at 2x for packed bf16; scalar_tensor_tensor measured 1x)
            vb = (
                vrep_bf[:, gs, :]
                .broadcast_to([128, GB, CD, tpb])
                .rearrange("p g c t -> p g t c")
            )
            nc.vector.tensor_mul(w4[:, gs, :, :], u4[:, gs, :, :], vb)
            wg = w5[:, ts, :, :]
            nc.vector.tensor_add(wg[:, :, :, 0:8], wg[:, :, :, 0:8], wg[:, :, :, 8:16])
            nc.vector.tensor_add(wg[:, :, :, 0:4], wg[:, :, :, 0:4], wg[:, :, :, 4:8])
            nc.vector.tensor_add(wg[:, :, :, 0:2], wg[:, :, :, 0:2], wg[:, :, :, 2:4])
            bg = b3[:, ts, :]
            # d-collapsed operands are stride-16 (1x on DVE): use idle GPSIMD
            if i == 1:
                nc.gpsimd.tensor_add(bg, wg[:, :, :, 0], wg[:, :, :, 1])
            else:
                nc.gpsimd.tensor_add(wg[:, :, :, 0], wg[:, :, :, 0], wg[:, :, :, 1])
                nc.gpsimd.tensor_add(bg, bg, wg[:, :, :, 0])
            # softmax over capsules (no max-subtraction: |b| is small)
            fs = slice(gr * TG * C, (gr + 1) * TG * C)
            e_g = e_exp[:, fs]
            nc.scalar.activation(e_g, b_log[:, fs], ACTF.Exp)
            # Z = sum_c e via a 5-2-1 add tree (tensor_reduce has no perf mode)
            et = e3[:, ts, :]
            z5 = zscr[:, ts, 0:5]
            z2 = zscr[:, ts, 5:7]
            z1 = zscr[:, ts, 7:8]
            _stt(nc.vector, z5, et[:, :, 0:5], et[:, :, 5:10], ALU.add)
            _stt(nc.vector, z2, z5[:, :, 0:2], z5[:, :, 2:4], ALU.add)
            _stt(nc.vector, z1, z2[:, :, 0:1], z2[:, :, 1:2], ALU.add)
            _stt(nc.vector, z1, z1, z5[:, :, 4:5], ALU.add)
            ss_g = ssum[:, ts]
            nc.vector.reciprocal(ss_g, zscr[:, ts, 7])
            rb = ss_g.broadcast_to([128, TG, C])
            # broadcast-innermost runs at 1x on DVE; park it on idle GPSIMD
            nc.gpsimd.tensor_mul(c3[:, ts, :], et, rb)

        def rt12_b(gr, i):
            # s[c,d] = sum_n c*u via per-tile matmuls with c stationary,
            # two batches packed per slot on col groups 0 / 32
            g0 = gr * GB
            sp = ps_S.tile([128, GB, CD], F32, tag="psS")
            pairs = [(2 * j, 2 * j + 1) for j in range(GB // 2)]
            for ja, jb in pairs:
                ga, gb = g0 + ja, g0 + jb
                pp = ps_P.tile([42, CD], F32, tag="psP")
                for tt in range(tpb):
                    nc.tensor.matmul(
                        pp[0:C, :],
                        lhsT=c_sm[:, (ga * tpb + tt) * C : (ga * tpb + tt + 1) * C],
                        rhs=u_hat[:, ga * tpb + tt, :],
                        start=(tt == 0),
                        stop=(tt == tpb - 1),
                        tile_position=(0, 0),
                    )
                    nc.tensor.matmul(
                        pp[32 : 32 + C, :],
                        lhsT=c_sm[:, (gb * tpb + tt) * C : (gb * tpb + tt + 1) * C],
                        rhs=u_hat[:, gb * tpb + tt, :],
                        start=(tt == 0),
                        stop=(tt == tpb - 1),
                        tile_position=(0, 32),
                    )
                pm = pool_sm.tile([42, CD], BF16, tag="pm")
                nc.vector.tensor_mul(pm[0:C, :], pp[0:C, :], mask[0:C, :])
                nc.vector.tensor_mul(
                    pm[32 : 32 + C, :], pp[32 : 32 + C, :], mask[32 : 32 + C, :]
                )
                nc.tensor.matmul(
                    sp[:, ja, :],
                    lhsT=ones_m[0:C, :],
                    rhs=pm[0:C, :],
                    start=True,
                    stop=True,
                )
                nc.tensor.matmul(
                    sp[:, jb, :],
                    lhsT=ones_m[32 : 32 + C, :],
                    rhs=pm[32 : 32 + C, :],
                    start=True,
                    stop=True,
                    tile_position=(32, 0),
                )
            squash_group(gr, sp, last=(i == 2))

        # ---------------- interleaved emission ----------------
        for gr in range(NG):
            prev = None
            for t in range(gr * TG, (gr + 1) * TG):
                cur = emit_trans(t)
                if prev is not None:
                    emit_gemm(prev[0], prev[1])
                prev = (t, cur)
            emit_gemm(prev[0], prev[1])
            if gr >= 1:
                rt12_a(gr - 1, 1)
            if gr >= 2:
                rt12_a(gr - 2, 2)
            rt0(gr)
            if gr >= 1:
                rt12_b(gr - 1, 1)
            if gr >= 2:
                rt12_b(gr - 2, 2)
        rt12_a(NG - 1, 1)
        rt12_b(NG - 1, 1)
        if NG >= 2:
            rt12_a(NG - 2, 2)
            rt12_b(NG - 2, 2)
        rt12_a(NG - 1, 2)
        rt12_b(NG - 1, 2)


# ----------------------------------------------------------------------------
_NC_CACHE = {}


def _get_nc():
    key = (BSH, N)
    if key not in _NC_CACHE:
        _NC_CACHE[key] = build_core_program()
    return _NC_CACHE[key]


def _run(x, W, **kw):
    from concourse.bass_utils import run_bass_kernel_spmd

    import ml_dtypes

    nc = _get_nc()
    x = np.ascontiguousarray(x, dtype=np.float32)
    W = np.ascontiguousarray(W, dtype=np.float32)
    ident = np.eye(128, dtype=ml_dtypes.bfloat16)
    mask1 = np.kron(np.eye(C, dtype=np.float32), np.ones((1, D), np.float32))
    mask = np.zeros((42, CD), np.float32)
    mask[0:C] = mask1
    mask[32 : 32 + C] = mask1
    mask = mask.astype(ml_dtypes.bfloat16)
    shards = x.reshape(NCORES, BSH * N, K)
    in_maps = [
        {"x": shards[c], "W": W, "ident": ident, "mask": mask} for c in range(NCORES)
    ]
    res = run_bass_kernel_spmd(nc, in_maps, core_ids=list(range(NCORES)), **kw)
    v = np.concatenate(
        [res.results[c]["v"].reshape(BSH, C, D) for c in range(NCORES)], axis=0
    )
    return v, res


def kernel(x, W):
    v, _ = _run(x, W)
    return v


def kernel_timed(x, W):
    v, res = _run(x, W, trace=True)
    return v, res.exec_time_ns


def kernel_traced(x, W):
    v, res = _run(x, W, trace=True)
    return v, res
